# revision 31
# baseline (speedup 1.0000x reference)
"""Trainium2 Bass kernel for CE-loss with spatially-varying label smoothing (SVLS).

Strategy (8 NeuronCores):
  - Shard over (n, z): core i handles n = i//4, z-slab [16*(i%4), 16*(i%4)+16),
    processed as 2 chunks of 8 z-slices; each chunk in 2 sub-chunks of 4
    z-slices sized to PSUM (7 T banks + 1 su bank). Stencil z-halo from
    host-side slab slicing; x/y halos from host-side edge padding + parity
    copies.
  - Engine split:
      DVE:     class masks (is_equal, 4x), per-tap d = I(v+d)-I(v) subs, the
               per-tap mask*u products (2x bf16), dxa/p2/pc products + class
               sum trees, f32 epilogue.
      ACT:     u = exp(-d^2/2) via one Derivative_Erf op per tap, PSUM
               drains, lse exp/ln, scalar chains.
      TensorE: tap accumulations as scaled-identity matmuls accumulating in
               PSUM (r2-group weight in the stationary): T_c = sum_k c_k u_k
               M_c (7 banks), su = sum_k c_k u_k (1 bank).
      GPSIMD:  exp-sum tree for lse, xc += x0 (kept light: GPSIMD shares the
               SBUF port with DVE).
  - Tile engines run in order, so the drain-dependent tails are software
    pipelined: emission order per core is
      P(0) T(0,0) D(0,0) T(0,1) tail(0,0) D(0,1) tail(0,1) P(1) T(1,0)
      epi(0) D(1,0) T(1,1) tail(1,0) D(1,1) tail(1,1) epi(1)
    so DVE never sits behind a PSUM drain.
  - Closed form of the reference's double normalization (unchanged):
      loss_voxel = lse - [ (A - uc*xc)/su + ns*xc ] / D,
      ns = 1 - uc/su + 1e-6, D = 2*ns - 1e-6,
      A = x0*su + sum_{c>=1} (x_c-x_0)*T_c, xc = x(v, lab(v)).
  - Per-core partial sums [128,2] f32 go back to host; host sums / N.
"""

import sys
import math

sys.path.insert(0, "/opt/trn_rl_repo")

import numpy as np
import ml_dtypes

import concourse.bass as bass
import concourse.bacc as bacc
import concourse.tile as tile
from concourse import mybir
from concourse.ap import AP as APc
from concourse.bass_utils import run_bass_kernel_spmd

dt = mybir.dt
BF16 = ml_dtypes.bfloat16
AF = mybir.ActivationFunctionType
OP = mybir.AluOpType

N, C, ZF, XF, YF = 2, 8, 64, 128, 128
NCORES = 8
ZSLAB = 16          # z-slices per core
ZCH = 8             # z-slices per chunk
NCH = ZSLAB // ZCH  # chunks per core
Z4 = 4              # z-slices per PSUM sub-chunk
NSC = ZCH // Z4     # sub-chunks per chunk

UC = 1.0 / (4.0 * math.pi * math.pi)           # center bilateral weight (const)
LNC2 = -2.0 * math.log(2.0 * math.pi)          # ln(C^2)
DERF_C = math.sqrt(math.pi) / 2.0              # DErf(x) = (2/sqrt(pi)) e^{-x^2}
# identity scales: 0 = center (UC), 1..3 = e^{lnC^2 - r2/2} (DErf factor
# compensated)
CK = [UC] + [math.exp(LNC2 - 0.5 * r2) * DERF_C for r2 in (1, 2, 3)]

RSQRT2 = float(1.0 / math.sqrt(2.0))


def _reg_const(nc, val, dtype=dt.float32):
    key = (dtype, val)
    if key in nc.const_aps.aps:
        return
    t = nc.alloc_sbuf_tensor(f"uconst-{dtype.name}-{val}", [128, 1], dtype)
    nc.gpsimd.memset(t.ap(), val)
    nc.const_aps.aps[key] = t.ap()


def _build():
    nc = bacc.Bacc(None)
    _reg_const(nc, float(UC))
    _reg_const(nc, 0.0)
    nc.all_engine_barrier()

    lab_d = nc.declare_dram_parameter("LAB", [NCH, 3, 2, 128, ZCH + 2, 132], dt.bfloat16, isOutput=False)
    img_d = nc.declare_dram_parameter("IMG", [NCH, 3, 2, 128, ZCH + 2, 132], dt.bfloat16, isOutput=False)
    x_d = nc.declare_dram_parameter("X", [NCH, 128, C, ZCH, 128], dt.bfloat16, isOutput=False)
    idt_d = nc.declare_dram_parameter("IDT", [128, 4, 128], dt.bfloat16, isOutput=False)
    red_d = nc.declare_dram_parameter("red", [128, NCH], dt.float32, isOutput=True)

    with tile.TileContext(nc) as tc:
        with (
            tc.tile_pool(name="pid", bufs=1) as pid,
            tc.tile_pool(name="pin", bufs=1) as pin,
            tc.tile_pool(name="pm", bufs=1) as pm,
            tc.tile_pool(name="pu", bufs=3) as pu,
            tc.tile_pool(name="pw", bufs=4) as pw,
            tc.tile_pool(name="pT", bufs=1) as pT,
            tc.tile_pool(name="pe", bufs=1) as pe,
            tc.tile_pool(name="pout", bufs=1) as pout,
            tc.tile_pool(name="psum", bufs=1, space="PSUM") as psum,
        ):
            idt = pid.tile([128, 4, 128], dt.bfloat16, name="idt")
            nc.sync.dma_start(idt[:], idt_d[:])
            idta = [idt[:, k] for k in range(4)]

            red = pout.tile([128, NCH], dt.float32, name="red")

            def bcast7(ap, nz=Z4):
                return ap.rearrange("p (o z) y -> p o z y", o=1).broadcast_to([128, C - 1, nz, 128])

            def emit_prologue(ch):
                st = {"ch": ch}
                labt, imgt = {}, {}
                for dxi in (1, 0, 2):
                    lt = pin.tile([128, ZCH + 2, 132], dt.bfloat16, tag=f"lab{dxi}1", name=f"lab{dxi}1")
                    nc.sync.dma_start(lt[:], lab_d[ch, dxi, 0])
                    labt[dxi] = lt
                    for par in (1, 2):
                        it = pin.tile([128, ZCH + 2, 132], dt.bfloat16, tag=f"img{dxi}{par}", name=f"img{dxi}{par}")
                        nc.sync.dma_start(it[:], img_d[ch, dxi, par - 1])
                        imgt[dxi, par] = it
                xt = pin.tile([128, C, ZCH, 128], dt.bfloat16, tag="xt", name="xt")
                nc.sync.dma_start(xt[:], x_d[ch])
                st["labt"], st["imgt"], st["xt"] = labt, imgt, xt
                st["su"] = pe.tile([128, ZCH, 128], dt.bfloat16, tag=f"su{ch}", name="su_bf")
                st["xc"] = pe.tile([128, ZCH, 128], dt.bfloat16, tag=f"xc{ch}", name="xc")
                st["A"] = pe.tile([128, ZCH, 128], dt.bfloat16, tag=f"A{ch}", name="A_bf")
                return st

            def emit_mid(st):
                # dxa / lse construction, emitted inside the pipelined region
                # (not at the serial chunk boundary)
                ch, xt = st["ch"], st["xt"]
                dxa = pT.tile([128, C - 1, ZCH, 128], dt.bfloat16, tag="dxa", name="dxa")
                nc.vector.tensor_tensor(
                    dxa[:], xt[:, 1:C],
                    xt[:, 0].rearrange("p (o z) y -> p o z y", o=1).broadcast_to([128, C - 1, ZCH, 128]),
                    OP.subtract)
                st["dxa"] = dxa
                ex = pe.tile([128, C, ZCH, 128], dt.bfloat16, tag="ex", name="ex")
                nc.scalar.activation(ex[:], xt[:], AF.Exp)
                nc.gpsimd.tensor_tensor(ex[:, 0:4], ex[:, 0:4], ex[:, 4:8], OP.add)
                nc.gpsimd.tensor_tensor(ex[:, 0:2], ex[:, 0:2], ex[:, 2:4], OP.add)
                nc.gpsimd.tensor_tensor(ex[:, 0], ex[:, 0], ex[:, 1], OP.add)
                lse = pe.tile([128, ZCH, 128], dt.bfloat16, tag=f"lse{ch}", name="lse")
                nc.scalar.activation(lse[:], ex[:, 0], AF.Ln)
                st["lse"] = lse

            def emit_taps(st, sc):
                z0 = Z4 * sc
                labt, imgt = st["labt"], st["imgt"]
                Tps = [psum.tile([128, 512], dt.float32, tag=f"T{c}", name=f"T{c}")
                       for c in range(C - 1)]
                SUps = psum.tile([128, 512], dt.float32, tag="su", name="SUps")
                ntap = 0
                for dx in (0, -1, 1):
                    dxi = 1 + dx
                    # center-group par-1 masks are read again by the deferred
                    # tail (pc), which is emitted after the NEXT sub-chunk's
                    # taps -> give them per-sc tags; everything else can share.
                    mtag = f"c{sc}" if dx == 0 else "s"
                    M1 = pm.tile([128, C - 1, 6, 132], dt.bfloat16, tag=f"M1{mtag}", name="M1")
                    for c in range(1, C):
                        nc.vector.tensor_scalar(
                            M1[:, c - 1], labt[dxi][:, z0:z0 + 6, :], float(c), None, OP.is_equal)
                    M2 = pm.tile([128, C - 1, 6, 132], dt.bfloat16,
                                 tag="M2c" if dx == 0 else "M2s", name="M2")
                    nc.sync.dma_start(M2[:, :, :, 2:132], M1[:, :, :, 1:131])
                    M = {1: M1, 2: M2}
                    if dx == 0:
                        st["Mcen", sc] = M1
                        # center tap: T_c = UC * M_c0 (starts the accum)
                        for c in range(C - 1):
                            nc.tensor.matmul(Tps[c][:], idta[0],
                                             M1[:, c, 1:1 + Z4, 2:130],
                                             start=True, stop=False)
                    imgC = imgt[1, 1][:, 1 + z0:1 + z0 + Z4, 2:130]
                    # dy = 0 first: its taps run off M1 while the M2 parity
                    # DMA is still in flight.
                    for dy in (0, -1, 1):
                        par = 1 if dy == 0 else 2
                        dzs = [dz for dz in (-1, 0, 1)
                               if not (dx == 0 and dz == 0 and dy == 0)]
                        nd = len(dzs)
                        zstep = dzs[1] - dzs[0]
                        ylo = par + 1 + dy
                        # batched d/u over the dz set via an overlapping-window
                        # AP (dz and z share the row stride)
                        src = imgt[dxi, par][:]
                        win = APc(src.tensor, (1 + z0 + dzs[0]) * 132 + ylo,
                                  [[1320, 128], [132 * zstep, nd], [132, Z4], [1, 128]])
                        d3 = pu.tile([128, 3, Z4, 128], dt.bfloat16, tag="d", name="d3", bufs=2)
                        nc.vector.tensor_tensor(
                            d3[:, 0:nd], win,
                            imgC.rearrange("p (o z) y -> p o z y", o=1)
                                .broadcast_to([128, nd, Z4, 128]),
                            OP.subtract)
                        u3 = pu.tile([128, 3, Z4, 128], dt.bfloat16, tag="u", name="u3", bufs=2)
                        nc.scalar.activation(u3[:, 0:nd], d3[:, 0:nd],
                                             AF.Derivative_Erf, scale=RSQRT2)
                        for i, dz in enumerate(dzs):
                            r2 = dz * dz + dx * dx + dy * dy
                            ntap += 1
                            prod = pw.tile([128, C - 1, Z4, 128], dt.bfloat16, tag="prod", name="prod")
                            nc.vector.tensor_tensor(
                                prod[:], M[par][:, :, 1 + dz:1 + dz + Z4,
                                                ylo:ylo + 128],
                                bcast7(u3[:, i]), OP.mult)
                            for c in range(C - 1):
                                nc.tensor.matmul(Tps[c][:], idta[r2], prod[:, c],
                                                 start=False, stop=(ntap == 26))
                            nc.tensor.matmul(SUps[:], idta[r2], u3[:, i],
                                             start=(ntap == 1), stop=(ntap == 26))
                st["Tps", sc], st["SUps", sc] = Tps, SUps

            def emit_drains(st, sc):
                z0 = Z4 * sc
                nc.scalar.activation(st["su"][:, z0:z0 + Z4], st["SUps", sc][:],
                                     AF.Identity, bias=float(UC))
                Tsb = pT.tile([128, C - 1, Z4, 128], dt.bfloat16, tag="Tsb", name="Tsb")
                for c in range(C - 1):
                    nc.scalar.copy(Tsb[:, c], st["Tps", sc][c][:])
                st["Tsb", sc] = Tsb

            def ctree(dst, P, extra):
                # dst = sum over class dim of P[:, 0:7] (+ extra)
                q3 = pw.tile([128, 3, Z4, 128], dt.bfloat16, tag="q3", name="q3", bufs=1)
                nc.vector.tensor_add(q3[:], P[:, 0:3], P[:, 3:6])
                nc.vector.tensor_add(dst, q3[:, 0], q3[:, 1])
                nc.vector.tensor_add(dst, dst, q3[:, 2])
                nc.vector.tensor_add(dst, dst, P[:, 6])
                if extra is not None:
                    nc.vector.tensor_add(dst, dst, extra)

            def emit_tail(st, sc):
                z0 = Z4 * sc
                xt, dxa = st["xt"], st["dxa"]
                # A' = sum_c dxa_c * T_c  (the x0*su term of A cancels against
                # r = 1/su in the epilogue: A/su = x0 + A'/su)
                p2 = pw.tile([128, C - 1, Z4, 128], dt.bfloat16, tag="prod", name="p2")
                nc.vector.tensor_tensor(p2[:], dxa[:, :, z0:z0 + Z4], st["Tsb", sc][:], OP.mult)
                ctree(st["A"][:, z0:z0 + Z4], p2, None)
                # xcd = sum_c dxa_c * M_c0(center)  (= xc - x0)
                pc = pw.tile([128, C - 1, Z4, 128], dt.bfloat16, tag="prod", name="pc")
                nc.vector.tensor_tensor(pc[:], st["Mcen", sc][:, :, 1:1 + Z4, 2:130],
                                        dxa[:, :, z0:z0 + Z4], OP.mult)
                ctree(st["xc"][:, z0:z0 + Z4], pc, None)
                # lse -= xc = lse - x0 - xcd  (off the epilogue's critical chain)
                nc.vector.tensor_tensor(st["lse"][:, z0:z0 + Z4], st["lse"][:, z0:z0 + Z4],
                                        xt[:, 0, z0:z0 + Z4], OP.subtract)
                nc.vector.tensor_tensor(st["lse"][:, z0:z0 + Z4], st["lse"][:, z0:z0 + Z4],
                                        st["xc"][:, z0:z0 + Z4], OP.subtract)

            def emit_epilogue(st):
                # loss_v = (lse - xc) - (A'*r - xcd) * rw
                #   r = 1/su, w = (2+1e-6) - 2*UC*r, rw = 1/w
                # st["lse"] already holds lse - xc; st["xc"] holds xcd = xc-x0.
                ch = st["ch"]
                lse, su_bf, xcd, A_bf = st["lse"], st["su"], st["xc"], st["A"]
                suf = pe.tile([128, ZCH, 128], dt.float32, tag="suf", name="suf")
                nc.scalar.copy(suf[:], su_bf[:])
                rsu = pe.tile([128, ZCH, 128], dt.float32, tag="rsu", name="rsu")
                nc.vector.reciprocal_approx_fast(rsu[:], suf[:])
                Dv = pe.tile([128, ZCH, 128], dt.float32, tag="Dv", name="Dv")
                nc.scalar.activation(Dv[:], rsu[:], AF.Copy, bias=float(2.0 + 1e-6),
                                     scale=float(-2.0 * UC))
                q = pe.tile([128, ZCH, 128], dt.float32, tag="suf", name="q")
                nc.vector.tensor_tensor(q[:], A_bf[:], rsu[:], OP.mult)
                rD = pe.tile([128, ZCH, 128], dt.float32, tag="rsu", name="rD")
                nc.vector.reciprocal_approx_fast(rD[:], Dv[:])
                nc.vector.tensor_tensor(q[:], q[:], xcd[:], OP.subtract)
                nc.vector.tensor_tensor(q[:], q[:], rD[:], OP.mult)
                nc.vector.tensor_tensor(q[:], lse[:], q[:], OP.subtract)
                nc.vector.tensor_reduce(red[:, ch:ch + 1], q[:], mybir.AxisListType.XY, OP.add)

            # software-pipelined emission (see module docstring)
            st0 = emit_prologue(0)
            emit_taps(st0, 0)
            emit_mid(st0)
            emit_drains(st0, 0)
            emit_taps(st0, 1)
            emit_tail(st0, 0)
            emit_drains(st0, 1)
            emit_tail(st0, 1)
            st1 = emit_prologue(1)
            emit_taps(st1, 0)
            emit_mid(st1)
            emit_epilogue(st0)
            emit_drains(st1, 0)
            emit_taps(st1, 1)
            emit_tail(st1, 0)
            emit_drains(st1, 1)
            emit_tail(st1, 1)
            emit_epilogue(st1)

            nc.sync.dma_start(red_d[:], red[:])
    nc.finalize()
    return nc


_NC = None


def _get_nc():
    global _NC
    if _NC is None:
        _NC = _build()
    return _NC


def _prep_inputs(inputs, labels, images):
    img = images[:, 1].astype(BF16)                      # [n,z,x,y] bf16
    lab = labels.astype(BF16)
    pad = ((0, 0), (1, 1), (1, 1), (1, 1))
    imgP = np.pad(img, pad, mode="edge")                  # [n,66,130,130]
    labP = np.pad(lab, pad, mode="edge")
    xb = inputs.astype(BF16)                              # [n,8,z,x,y]

    IDT = np.zeros((128, 4, 128), BF16)
    for k, cval in enumerate(CK):
        IDT[np.arange(128), k, np.arange(128)] = BF16(cval)

    in_maps = []
    for core in range(NCORES):
        n, q = core // 4, core % 4
        z0 = ZSLAB * q
        LAB = np.zeros((NCH, 3, 2, 128, ZCH + 2, 132), BF16)
        IMG = np.zeros((NCH, 3, 2, 128, ZCH + 2, 132), BF16)
        X = np.zeros((NCH, 128, C, ZCH, 128), BF16)
        for ch in range(NCH):
            for dxi in range(3):
                labs = labP[n, z0 + ZCH * ch: z0 + ZCH * ch + ZCH + 2, dxi: dxi + 128, :]
                imgs = imgP[n, z0 + ZCH * ch: z0 + ZCH * ch + ZCH + 2, dxi: dxi + 128, :]
                labs = labs.transpose(1, 0, 2)            # [128, ZCH+2, 130]
                imgs = imgs.transpose(1, 0, 2)
                for par in (1, 2):
                    LAB[ch, dxi, par - 1, :, :, par: par + 130] = labs
                    IMG[ch, dxi, par - 1, :, :, par: par + 130] = imgs
            X[ch] = xb[n, :, z0 + ZCH * ch: z0 + ZCH * ch + ZCH, :, :].transpose(2, 0, 1, 3)
        in_maps.append({"LAB": LAB, "IMG": IMG, "X": X, "IDT": IDT})
    return in_maps


def kernel(inputs: np.ndarray, labels: np.ndarray, images: np.ndarray) -> np.ndarray:
    in_maps = _prep_inputs(inputs, labels, images)
    nc = _get_nc()
    res = run_bass_kernel_spmd(nc, in_maps, list(range(NCORES)))
    total = np.float64(0.0)
    for core in range(NCORES):
        total += np.asarray(res.results[core]["red"], np.float64).sum()
    loss = total / float(N * ZF * XF * YF)
    return np.float32(loss)


# revision 33
# speedup vs baseline: 1.0044x; 1.0044x over previous
"""Trainium2 Bass kernel for CE-loss with spatially-varying label smoothing (SVLS).

Strategy (8 NeuronCores):
  - Shard over (n, z): core i handles n = i//4, z-slab [16*(i%4), 16*(i%4)+16),
    processed as 2 chunks of 8 z-slices; each chunk in 2 sub-chunks of 4
    z-slices sized to PSUM (7 T banks + 1 su bank). Stencil z-halo from
    host-side slab slicing; x/y halos from host-side edge padding + parity
    copies.
  - Engine split:
      DVE:     class masks (is_equal, 4x), per-tap d = I(v+d)-I(v) subs, the
               per-tap mask*u products (2x bf16), dxa/p2/pc products + class
               sum trees, f32 epilogue.
      ACT:     u = exp(-d^2/2) via one Derivative_Erf op per tap, PSUM
               drains, lse exp/ln, scalar chains.
      TensorE: tap accumulations as scaled-identity matmuls accumulating in
               PSUM (r2-group weight in the stationary): T_c = sum_k c_k u_k
               M_c (7 banks), su = sum_k c_k u_k (1 bank).
      GPSIMD:  exp-sum tree for lse, xc += x0 (kept light: GPSIMD shares the
               SBUF port with DVE).
  - Tile engines run in order, so the drain-dependent tails are software
    pipelined: emission order per core is
      P(0) T(0,0) D(0,0) T(0,1) tail(0,0) D(0,1) tail(0,1) P(1) T(1,0)
      epi(0) D(1,0) T(1,1) tail(1,0) D(1,1) tail(1,1) epi(1)
    so DVE never sits behind a PSUM drain.
  - Closed form of the reference's double normalization (unchanged):
      loss_voxel = lse - [ (A - uc*xc)/su + ns*xc ] / D,
      ns = 1 - uc/su + 1e-6, D = 2*ns - 1e-6,
      A = x0*su + sum_{c>=1} (x_c-x_0)*T_c, xc = x(v, lab(v)).
  - Per-core partial sums [128,2] f32 go back to host; host sums / N.
"""

import sys
import math

sys.path.insert(0, "/opt/trn_rl_repo")

import numpy as np
import ml_dtypes

import concourse.bass as bass
import concourse.bacc as bacc
import concourse.tile as tile
from concourse import mybir
from concourse.ap import AP as APc
from concourse.bass_utils import run_bass_kernel_spmd

dt = mybir.dt
BF16 = ml_dtypes.bfloat16
AF = mybir.ActivationFunctionType
OP = mybir.AluOpType

N, C, ZF, XF, YF = 2, 8, 64, 128, 128
NCORES = 8
ZSLAB = 16          # z-slices per core
ZCH = 8             # z-slices per chunk
NCH = ZSLAB // ZCH  # chunks per core
Z4 = 4              # z-slices per PSUM sub-chunk
NSC = ZCH // Z4     # sub-chunks per chunk

UC = 1.0 / (4.0 * math.pi * math.pi)           # center bilateral weight (const)
LNC2 = -2.0 * math.log(2.0 * math.pi)          # ln(C^2)
DERF_C = math.sqrt(math.pi) / 2.0              # DErf(x) = (2/sqrt(pi)) e^{-x^2}
# identity scales: 0 = center (UC), 1..3 = e^{lnC^2 - r2/2} (DErf factor
# compensated)
CK = [UC] + [math.exp(LNC2 - 0.5 * r2) * DERF_C for r2 in (1, 2, 3)]

RSQRT2 = float(1.0 / math.sqrt(2.0))


def _reg_const(nc, val, dtype=dt.float32):
    key = (dtype, val)
    if key in nc.const_aps.aps:
        return
    t = nc.alloc_sbuf_tensor(f"uconst-{dtype.name}-{val}", [128, 1], dtype)
    nc.gpsimd.memset(t.ap(), val)
    nc.const_aps.aps[key] = t.ap()


def _build():
    nc = bacc.Bacc(None)
    _reg_const(nc, float(UC))
    _reg_const(nc, 0.0)
    nc.all_engine_barrier()

    lab_d = nc.declare_dram_parameter("LAB", [NCH, 3, 2, 128, ZCH + 2, 132], dt.bfloat16, isOutput=False)
    img_d = nc.declare_dram_parameter("IMG", [NCH, 3, 2, 128, ZCH + 2, 132], dt.bfloat16, isOutput=False)
    x_d = nc.declare_dram_parameter("X", [NCH, 128, C, ZCH, 128], dt.bfloat16, isOutput=False)
    idt_d = nc.declare_dram_parameter("IDT", [128, 4, 128], dt.bfloat16, isOutput=False)
    red_d = nc.declare_dram_parameter("red", [128, NCH], dt.float32, isOutput=True)

    with tile.TileContext(nc) as tc:
        with (
            tc.tile_pool(name="pid", bufs=1) as pid,
            tc.tile_pool(name="pin", bufs=1) as pin,
            tc.tile_pool(name="pm", bufs=1) as pm,
            tc.tile_pool(name="pu", bufs=3) as pu,
            tc.tile_pool(name="pw", bufs=4) as pw,
            tc.tile_pool(name="pT", bufs=1) as pT,
            tc.tile_pool(name="pe", bufs=1) as pe,
            tc.tile_pool(name="pout", bufs=1) as pout,
            tc.tile_pool(name="psum", bufs=1, space="PSUM") as psum,
        ):
            idt = pid.tile([128, 4, 128], dt.bfloat16, name="idt")
            nc.sync.dma_start(idt[:], idt_d[:])
            idta = [idt[:, k] for k in range(4)]

            red = pout.tile([128, NCH], dt.float32, name="red")

            def bcast7(ap, nz=Z4):
                return ap.rearrange("p (o z) y -> p o z y", o=1).broadcast_to([128, C - 1, nz, 128])

            def emit_prologue(ch):
                st = {"ch": ch}
                labt, imgt = {}, {}
                for dxi in (1, 0, 2):
                    lt = pin.tile([128, ZCH + 2, 132], dt.bfloat16, tag=f"lab{dxi}1", name=f"lab{dxi}1")
                    nc.sync.dma_start(lt[:], lab_d[ch, dxi, 0])
                    labt[dxi] = lt
                    for par in (1, 2):
                        it = pin.tile([128, ZCH + 2, 132], dt.bfloat16, tag=f"img{dxi}{par}", name=f"img{dxi}{par}")
                        nc.sync.dma_start(it[:], img_d[ch, dxi, par - 1])
                        imgt[dxi, par] = it
                xt = pin.tile([128, C, ZCH, 128], dt.bfloat16, tag="xt", name="xt")
                nc.sync.dma_start(xt[:], x_d[ch])
                st["labt"], st["imgt"], st["xt"] = labt, imgt, xt

                dxa = pT.tile([128, C - 1, ZCH, 128], dt.bfloat16, tag="dxa", name="dxa")
                nc.vector.tensor_tensor(
                    dxa[:], xt[:, 1:C],
                    xt[:, 0].rearrange("p (o z) y -> p o z y", o=1).broadcast_to([128, C - 1, ZCH, 128]),
                    OP.subtract)
                st["dxa"] = dxa

                # lse = ln(sum_c exp(x_c)); wide exp on ACT, sum tree on DVE
                # (GPSIMD shares the DVE SBUF port — running the tree there
                # stalls DVE prods more than the tree costs on DVE itself)
                ex = pe.tile([128, C, ZCH, 128], dt.bfloat16, tag="ex", name="ex")
                nc.scalar.activation(ex[:], xt[:], AF.Exp)
                nc.gpsimd.tensor_tensor(ex[:, 0:4], ex[:, 0:4], ex[:, 4:8], OP.add)
                nc.gpsimd.tensor_tensor(ex[:, 0:2], ex[:, 0:2], ex[:, 2:4], OP.add)
                nc.gpsimd.tensor_tensor(ex[:, 0], ex[:, 0], ex[:, 1], OP.add)
                lse = pe.tile([128, ZCH, 128], dt.bfloat16, tag=f"lse{ch}", name="lse")
                nc.scalar.activation(lse[:], ex[:, 0], AF.Ln)
                st["lse"] = lse

                st["su"] = pe.tile([128, ZCH, 128], dt.bfloat16, tag=f"su{ch}", name="su_bf")
                st["xc"] = pe.tile([128, ZCH, 128], dt.bfloat16, tag=f"xc{ch}", name="xc")
                st["A"] = pe.tile([128, ZCH, 128], dt.bfloat16, tag=f"A{ch}", name="A_bf")
                return st

            def emit_taps(st, sc):
                z0 = Z4 * sc
                labt, imgt = st["labt"], st["imgt"]
                Tps = [psum.tile([128, 512], dt.float32, tag=f"T{c}", name=f"T{c}")
                       for c in range(C - 1)]
                SUps = psum.tile([128, 512], dt.float32, tag="su", name="SUps")
                ntap = 0
                for dx in (0, -1, 1):
                    dxi = 1 + dx
                    # center-group par-1 masks are read again by the deferred
                    # tail (pc), which is emitted after the NEXT sub-chunk's
                    # taps -> give them per-sc tags; everything else can share.
                    mtag = f"c{sc}" if dx == 0 else "s"
                    M1 = pm.tile([128, C - 1, 6, 132], dt.bfloat16, tag=f"M1{mtag}", name="M1")
                    for c in range(1, C):
                        nc.vector.tensor_scalar(
                            M1[:, c - 1], labt[dxi][:, z0:z0 + 6, :], float(c), None, OP.is_equal)
                    M2 = pm.tile([128, C - 1, 6, 132], dt.bfloat16,
                                 tag="M2c" if dx == 0 else "M2s", name="M2")
                    nc.sync.dma_start(M2[:, :, :, 2:132], M1[:, :, :, 1:131])
                    M = {1: M1, 2: M2}
                    if dx == 0:
                        st["Mcen", sc] = M1
                        # center tap: T_c = UC * M_c0 (starts the accum)
                        for c in range(C - 1):
                            nc.tensor.matmul(Tps[c][:], idta[0],
                                             M1[:, c, 1:1 + Z4, 2:130],
                                             start=True, stop=False)
                    imgC = imgt[1, 1][:, 1 + z0:1 + z0 + Z4, 2:130]
                    # dy = 0 first: its taps run off M1 while the M2 parity
                    # DMA is still in flight.
                    for dy in (0, -1, 1):
                        par = 1 if dy == 0 else 2
                        dzs = [dz for dz in (-1, 0, 1)
                               if not (dx == 0 and dz == 0 and dy == 0)]
                        nd = len(dzs)
                        zstep = dzs[1] - dzs[0]
                        ylo = par + 1 + dy
                        # batched d/u over the dz set via an overlapping-window
                        # AP (dz and z share the row stride)
                        src = imgt[dxi, par][:]
                        win = APc(src.tensor, (1 + z0 + dzs[0]) * 132 + ylo,
                                  [[1320, 128], [132 * zstep, nd], [132, Z4], [1, 128]])
                        d3 = pu.tile([128, 3, Z4, 128], dt.bfloat16, tag="d", name="d3", bufs=2)
                        nc.vector.tensor_tensor(
                            d3[:, 0:nd], win,
                            imgC.rearrange("p (o z) y -> p o z y", o=1)
                                .broadcast_to([128, nd, Z4, 128]),
                            OP.subtract)
                        u3 = pu.tile([128, 3, Z4, 128], dt.bfloat16, tag="u", name="u3", bufs=2)
                        nc.scalar.activation(u3[:, 0:nd], d3[:, 0:nd],
                                             AF.Derivative_Erf, scale=RSQRT2)
                        for i, dz in enumerate(dzs):
                            r2 = dz * dz + dx * dx + dy * dy
                            ntap += 1
                            prod = pw.tile([128, C - 1, Z4, 128], dt.bfloat16, tag="prod", name="prod")
                            nc.vector.tensor_tensor(
                                prod[:], M[par][:, :, 1 + dz:1 + dz + Z4,
                                                ylo:ylo + 128],
                                bcast7(u3[:, i]), OP.mult)
                            # su first: it needs only u, so TensorE runs it
                            # while DVE is still producing this tap's prod
                            nc.tensor.matmul(SUps[:], idta[r2], u3[:, i],
                                             start=(ntap == 1), stop=(ntap == 26))
                            for c in range(C - 1):
                                nc.tensor.matmul(Tps[c][:], idta[r2], prod[:, c],
                                                 start=False, stop=(ntap == 26))
                st["Tps", sc], st["SUps", sc] = Tps, SUps

            def emit_drains(st, sc):
                z0 = Z4 * sc
                nc.scalar.activation(st["su"][:, z0:z0 + Z4], st["SUps", sc][:],
                                     AF.Identity, bias=float(UC))
                Tsb = pT.tile([128, C - 1, Z4, 128], dt.bfloat16, tag="Tsb", name="Tsb")
                for c in range(C - 1):
                    nc.scalar.copy(Tsb[:, c], st["Tps", sc][c][:])
                st["Tsb", sc] = Tsb

            def ctree(dst, P, extra):
                # dst = sum over class dim of P[:, 0:7] (+ extra)
                q3 = pw.tile([128, 3, Z4, 128], dt.bfloat16, tag="q3", name="q3", bufs=1)
                nc.vector.tensor_add(q3[:], P[:, 0:3], P[:, 3:6])
                nc.vector.tensor_add(dst, q3[:, 0], q3[:, 1])
                nc.vector.tensor_add(dst, dst, q3[:, 2])
                nc.vector.tensor_add(dst, dst, P[:, 6])
                if extra is not None:
                    nc.vector.tensor_add(dst, dst, extra)

            def emit_tail(st, sc):
                z0 = Z4 * sc
                xt, dxa = st["xt"], st["dxa"]
                # A' = sum_c dxa_c * T_c  (the x0*su term of A cancels against
                # r = 1/su in the epilogue: A/su = x0 + A'/su)
                p2 = pw.tile([128, C - 1, Z4, 128], dt.bfloat16, tag="prod", name="p2")
                nc.vector.tensor_tensor(p2[:], dxa[:, :, z0:z0 + Z4], st["Tsb", sc][:], OP.mult)
                ctree(st["A"][:, z0:z0 + Z4], p2, None)
                # xcd = sum_c dxa_c * M_c0(center)  (= xc - x0)
                pc = pw.tile([128, C - 1, Z4, 128], dt.bfloat16, tag="prod", name="pc")
                nc.vector.tensor_tensor(pc[:], st["Mcen", sc][:, :, 1:1 + Z4, 2:130],
                                        dxa[:, :, z0:z0 + Z4], OP.mult)
                ctree(st["xc"][:, z0:z0 + Z4], pc, None)
                # lse -= xc = lse - x0 - xcd  (off the epilogue's critical chain)
                nc.vector.tensor_tensor(st["lse"][:, z0:z0 + Z4], st["lse"][:, z0:z0 + Z4],
                                        xt[:, 0, z0:z0 + Z4], OP.subtract)
                nc.vector.tensor_tensor(st["lse"][:, z0:z0 + Z4], st["lse"][:, z0:z0 + Z4],
                                        st["xc"][:, z0:z0 + Z4], OP.subtract)

            def emit_epilogue(st):
                # loss_v = (lse - xc) - (A'*r - xcd) * rw
                #   r = 1/su, w = (2+1e-6) - 2*UC*r, rw = 1/w
                # st["lse"] already holds lse - xc; st["xc"] holds xcd = xc-x0.
                ch = st["ch"]
                lse, su_bf, xcd, A_bf = st["lse"], st["su"], st["xc"], st["A"]
                suf = pe.tile([128, ZCH, 128], dt.float32, tag="suf", name="suf")
                nc.scalar.copy(suf[:], su_bf[:])
                rsu = pe.tile([128, ZCH, 128], dt.float32, tag="rsu", name="rsu")
                nc.vector.reciprocal_approx_fast(rsu[:], suf[:])
                Dv = pe.tile([128, ZCH, 128], dt.float32, tag="Dv", name="Dv")
                nc.scalar.activation(Dv[:], rsu[:], AF.Copy, bias=float(2.0 + 1e-6),
                                     scale=float(-2.0 * UC))
                q = pe.tile([128, ZCH, 128], dt.float32, tag="suf", name="q")
                nc.vector.tensor_tensor(q[:], A_bf[:], rsu[:], OP.mult)
                rD = pe.tile([128, ZCH, 128], dt.float32, tag="rsu", name="rD")
                nc.vector.reciprocal_approx_fast(rD[:], Dv[:])
                nc.vector.tensor_tensor(q[:], q[:], xcd[:], OP.subtract)
                nc.vector.tensor_tensor(q[:], q[:], rD[:], OP.mult)
                nc.vector.tensor_tensor(q[:], lse[:], q[:], OP.subtract)
                nc.vector.tensor_reduce(red[:, ch:ch + 1], q[:], mybir.AxisListType.XY, OP.add)

            # software-pipelined emission (see module docstring)
            st0 = emit_prologue(0)
            emit_taps(st0, 0)
            emit_drains(st0, 0)
            emit_taps(st0, 1)
            emit_tail(st0, 0)
            emit_drains(st0, 1)
            emit_tail(st0, 1)
            st1 = emit_prologue(1)
            emit_taps(st1, 0)
            emit_epilogue(st0)
            emit_drains(st1, 0)
            emit_taps(st1, 1)
            emit_tail(st1, 0)
            emit_drains(st1, 1)
            emit_tail(st1, 1)
            emit_epilogue(st1)

            nc.sync.dma_start(red_d[:], red[:])
    nc.finalize()
    return nc


_NC = None


def _get_nc():
    global _NC
    if _NC is None:
        _NC = _build()
    return _NC


def _prep_inputs(inputs, labels, images):
    img = images[:, 1].astype(BF16)                      # [n,z,x,y] bf16
    lab = labels.astype(BF16)
    pad = ((0, 0), (1, 1), (1, 1), (1, 1))
    imgP = np.pad(img, pad, mode="edge")                  # [n,66,130,130]
    labP = np.pad(lab, pad, mode="edge")
    xb = inputs.astype(BF16)                              # [n,8,z,x,y]

    IDT = np.zeros((128, 4, 128), BF16)
    for k, cval in enumerate(CK):
        IDT[np.arange(128), k, np.arange(128)] = BF16(cval)

    in_maps = []
    for core in range(NCORES):
        n, q = core // 4, core % 4
        z0 = ZSLAB * q
        LAB = np.zeros((NCH, 3, 2, 128, ZCH + 2, 132), BF16)
        IMG = np.zeros((NCH, 3, 2, 128, ZCH + 2, 132), BF16)
        X = np.zeros((NCH, 128, C, ZCH, 128), BF16)
        for ch in range(NCH):
            for dxi in range(3):
                labs = labP[n, z0 + ZCH * ch: z0 + ZCH * ch + ZCH + 2, dxi: dxi + 128, :]
                imgs = imgP[n, z0 + ZCH * ch: z0 + ZCH * ch + ZCH + 2, dxi: dxi + 128, :]
                labs = labs.transpose(1, 0, 2)            # [128, ZCH+2, 130]
                imgs = imgs.transpose(1, 0, 2)
                for par in (1, 2):
                    LAB[ch, dxi, par - 1, :, :, par: par + 130] = labs
                    IMG[ch, dxi, par - 1, :, :, par: par + 130] = imgs
            X[ch] = xb[n, :, z0 + ZCH * ch: z0 + ZCH * ch + ZCH, :, :].transpose(2, 0, 1, 3)
        in_maps.append({"LAB": LAB, "IMG": IMG, "X": X, "IDT": IDT})
    return in_maps


def kernel(inputs: np.ndarray, labels: np.ndarray, images: np.ndarray) -> np.ndarray:
    in_maps = _prep_inputs(inputs, labels, images)
    nc = _get_nc()
    res = run_bass_kernel_spmd(nc, in_maps, list(range(NCORES)))
    total = np.float64(0.0)
    for core in range(NCORES):
        total += np.asarray(res.results[core]["red"], np.float64).sum()
    loss = total / float(N * ZF * XF * YF)
    return np.float32(loss)
